# revision 14
# baseline (speedup 1.0000x reference)
"""GQA forward (b=2, s=2048, H=32 q heads, 8 kv heads, d=64) on 8 TRN2 cores.

Sharding: core k owns query heads 4k..4k+3 and kv head k. GQA group
structure makes attention fully local per core (q heads 4k..4k+3 attend
only to kv head k). x is replicated; W columns are sharded; outputs are
column-concatenated.

v2 layout (all matmul operands bf16; HW fp32r streams at 2-4 cyc/row while
bf16 streams at 1, and bf16 halves DMA bytes):
  - x is transposed + bf16-cast on the HOST; x.T tiles DMA straight into
    SBUF (kills the 512 PE transposes + 512 DVE evictions of v1).
  - Projections in natural layout: QKV[s,384] accumulated over 16 k-chunks.
  - RoPE on DVE (stride-2 free-dim views), output bf16.
  - Q/K flips via the DMA-transpose XBAR (16x128 tiles, bf16) instead of
    PE transposes: qn[:,0:128]->qta, qn[:,128:256]->qtb, qn[:,256:384]->
    kt_res rows 0:128 (V.T rows then overwritten by the kt dup DMA).
  - Attention in transposed layout: S.T[kv,q] = K @ Q.T per 128-kv block,
    two kv blocks share one PSUM tile so exp batches 2 strips per ACT
    instruction; causal via triangular predicated masks on diagonal
    blocks; ctx.T[80,q] = [V|1|0pad].T @ P.T accumulated in PSUM (row 64 =
    softmax sums, rows 65:80 zero pad so downstream reads are initialized).
  - Finalize: evict ctx.T to bf16 SBUF, DMA-transpose back to [q,80],
    normalize with a per-partition divide on GpSimd (Pool is otherwise
    idle), assemble [128,4,256] f32 per s-tile, one output DMA per s-tile.
"""

import numpy as np
from contextlib import ExitStack

import ml_dtypes

import concourse.bass as bass
import concourse.bacc as bacc
import concourse.mybir as mybir
from concourse import tile
from concourse.bass_utils import run_bass_kernel_spmd

F32 = mybir.dt.float32
BF16 = mybir.dt.bfloat16
U8 = mybir.dt.uint8
MUL = mybir.AluOpType.mult
ADD = mybir.AluOpType.add
DIV = mybir.AluOpType.divide
EXP = mybir.ActivationFunctionType.Exp

B = 2
S = 2048
DIN = 2048
D = 64              # head dim
HPC = 4             # query heads per core
NCORES = 8
WCOLS = 4 * D + D + D  # 256 q cols + 64 k + 64 v = 384
RC = 320            # roped columns (4 q heads + k head)
ST = 512            # s-tile (rows per outer step)
NST = B * S // ST   # 8 s-tiles
NCH = DIN // 128    # 16 k-chunks
NKV = S // 128      # kv tiles per batch
NEG = -30000.0      # pre-scale mask fill; exp(NEG/8) == 0 in f32


def build_bass():
    nc = bacc.Bacc(None, target_bir_lowering=False)
    xt_d = nc.declare_dram_parameter("xt", [DIN, B * S], BF16, isOutput=False)
    w_d = nc.declare_dram_parameter("w", [DIN, WCOLS], BF16, isOutput=False)
    cos_d = nc.declare_dram_parameter("cosn", [S, RC], F32, isOutput=False)
    sin_d = nc.declare_dram_parameter("sinn", [S, RC], F32, isOutput=False)
    mask_d = nc.declare_dram_parameter("mask", [128, 128], U8, isOutput=False)
    id_d = nc.declare_dram_parameter("ident", [128, 128], BF16, isOutput=False)
    out_d = nc.declare_dram_parameter("out", [B * S, HPC * D], F32, isOutput=True)

    with ExitStack() as ctx:
        tc = ctx.enter_context(tile.TileContext(nc))
        const = ctx.enter_context(tc.tile_pool(name="const", bufs=1))
        resid = ctx.enter_context(tc.tile_pool(name="resid", bufs=1))
        xt_p = ctx.enter_context(tc.tile_pool(name="xt", bufs=3))
        tab_p = ctx.enter_context(tc.tile_pool(name="tab", bufs=3))
        qn_p = ctx.enter_context(tc.tile_pool(name="qn", bufs=3))
        qt_p = ctx.enter_context(tc.tile_pool(name="qt", bufs=2))
        p_p = ctx.enter_context(tc.tile_pool(name="p", bufs=4))
        cxs_p = ctx.enter_context(tc.tile_pool(name="cxs", bufs=3))
        fo_p = ctx.enter_context(tc.tile_pool(name="fo", bufs=3))
        ob_p = ctx.enter_context(tc.tile_pool(name="ob", bufs=2))
        pr_ps = ctx.enter_context(tc.tile_pool(name="pr_ps", bufs=2, space="PSUM"))
        sc_ps = ctx.enter_context(tc.tile_pool(name="sc_ps", bufs=2, space="PSUM"))
        cx_ps = ctx.enter_context(tc.tile_pool(name="cx_ps", bufs=1, space="PSUM"))
        fi_ps = ctx.enter_context(tc.tile_pool(name="fi_ps", bufs=1, space="PSUM"))

        # constants / residents
        w_sb = const.tile([128, NCH, WCOLS], BF16)
        nc.sync.dma_start(
            out=w_sb[:], in_=w_d.rearrange("(c p) n -> p c n", p=128))
        mask_sb = const.tile([128, 128], U8)
        nc.sync.dma_start(out=mask_sb[:], in_=mask_d[:])
        ident = const.tile([128, 128], BF16)
        nc.sync.dma_start(out=ident[:], in_=id_d[:])
        neg_sb = const.tile([128, 128], F32)
        nc.vector.memset(neg_sb[:], NEG)

        # rows 0-63: K.T (RoPE'd); rows 64-127: duplicate copy so the scores
        # matmul lhsT can match either base partition of the Q halves
        kt_res = resid.tile([128, B * S], BF16)
        # [V | 1 | 0-pad] per kv tile: col 64 = ones (softmax sums land in
        # ctx.T row 64), cols 65:80 zero so ctx.T rows 65:80 read initialized
        vp_res = resid.tile([128, B * NKV, 80], BF16)
        nc.vector.memset(vp_res[:, :, 64:65], 1.0)
        nc.vector.memset(vp_res[:, :, 65:80], 0.0)

        xt_view = xt_d.rearrange("(c p) s -> p c s", p=128)

        for st in range(NST):
            b, sti = divmod(st, 4)

            # ---- x.T tiles straight from DRAM (host pre-transposed) ----
            xt = xt_p.tile([128, NCH, ST], BF16)
            nc.sync.dma_start(
                out=xt[:], in_=xt_view[:, :, st * ST:(st + 1) * ST])

            # ---- projections (natural layout) + RoPE + DMA-transpose ----
            qta = qt_p.tile([128, ST], BF16, tag="qta")   # heads 0,1 as [d,s]
            qtb = qt_p.tile([128, ST], BF16, tag="qtb")   # heads 2,3 as [d,s]
            ctab = tab_p.tile([128, 4, RC], F32, tag="ctab")
            nc.sync.dma_start(
                out=ctab[:],
                in_=cos_d[sti * ST:(sti + 1) * ST, :].rearrange(
                    "(q p) n -> p q n", p=128))
            stab = tab_p.tile([128, 4, RC], F32, tag="stab")
            nc.sync.dma_start(
                out=stab[:],
                in_=sin_d[sti * ST:(sti + 1) * ST, :].rearrange(
                    "(q p) n -> p q n", p=128))
            for pt in range(4):
                t = sti * 4 + pt  # within-batch 128-row block index
                pp = pr_ps.tile([128, WCOLS], F32, tag="pp")
                for c in range(NCH):
                    nc.tensor.matmul(
                        pp[:], xt[:, c, pt * 128:(pt + 1) * 128],
                        w_sb[:, c, :], start=(c == 0), stop=(c == NCH - 1))
                qn = qn_p.tile([128, WCOLS], BF16, tag="qn")
                ts = qn_p.tile([128, RC], BF16, tag="ts")
                # even cols: qe*c - qo*s ; odd cols: qo*c + qe*s
                nc.vector.scalar_tensor_tensor(
                    ts[:, 0:RC:2], pp[:, 1:RC:2], -1.0, stab[:, pt, 0:RC:2],
                    MUL, MUL)
                nc.vector.tensor_tensor(
                    ts[:, 1:RC:2], pp[:, 0:RC:2], stab[:, pt, 1:RC:2], MUL)
                nc.vector.tensor_tensor(qn[:, 0:RC], pp[:, 0:RC],
                                        ctab[:, pt, :], MUL)
                nc.vector.tensor_tensor(qn[:, 0:RC], qn[:, 0:RC], ts[:], ADD)
                # V columns: into the [V|1|0] resident (natural [kv, d])
                nc.vector.tensor_copy(
                    vp_res[:, b * NKV + t, 0:64], pp[:, RC:WCOLS])
                # and into qn so the K|V DMA-transpose reads initialized data
                nc.vector.tensor_copy(qn[:, RC:WCOLS], pp[:, RC:WCOLS])
                # flips via DMA-transpose XBAR
                nc.sync.dma_start(
                    out=qta[:, pt * 128:(pt + 1) * 128],
                    in_=qn[:, 0:128], transpose=True)
                nc.sync.dma_start(
                    out=qtb[:, pt * 128:(pt + 1) * 128],
                    in_=qn[:, 128:256], transpose=True)
                # rows 0:64 = K.T, rows 64:128 = V.T (overwritten by dup).
                # Issued on the Activation HWDGE queue to unload SP.
                nc.scalar.dma_start(
                    out=kt_res[:, st * ST + pt * 128:st * ST + (pt + 1) * 128],
                    in_=qn[:, 256:384], transpose=True)
            nc.sync.dma_start(
                out=kt_res[64:128, st * ST:(st + 1) * ST],
                in_=kt_res[0:64, st * ST:(st + 1) * ST])

            ob = ob_p.tile([128, 4, HPC * D], F32)

            # ---- attention for the 4 heads of this q-tile ----
            for h in range(HPC):
                p0 = (h % 2) * 64
                qh = (qta if h < 2 else qtb)[p0:p0 + 64, :]

                def kt(j):
                    return kt_res[p0:p0 + 64,
                                  b * S + j * 128:b * S + (j + 1) * 128]

                def vp(j):
                    return vp_res[:, b * NKV + j, :]

                cxt = cx_ps.tile([128, ST], F32, tag="cxt")
                first = True
                # full sub-diagonal blocks, two kv blocks per PSUM tile
                for jp in range(0, 4 * sti, 2):
                    sc = sc_ps.tile([128, 2, ST], F32, tag="sc")
                    for jj in (0, 1):
                        nc.tensor.matmul(sc[:, jj, :], kt(jp + jj), qh[:, :],
                                         start=True, stop=True)
                    psb = p_p.tile([128, 2, ST], BF16, tag="psb")
                    nc.scalar.activation(psb[:], sc[:], EXP, scale=0.125)
                    for jj in (0, 1):
                        nc.tensor.matmul(
                            cxt[0:80, :], vp(jp + jj), psb[:, jj, :],
                            start=first, stop=False)
                        first = False
                # diagonal strips r=0..3 (kv block 4*sti+r vs q cols
                # 128r:512), packed two per PSUM tile
                j0 = 4 * sti
                scd = sc_ps.tile([128, 2 * ST], F32, tag="sc")
                nc.tensor.matmul(scd[:, 0:512], kt(j0), qh[:, :],
                                 start=True, stop=True)
                nc.tensor.matmul(scd[:, 512:896], kt(j0 + 1), qh[:, 128:512],
                                 start=True, stop=True)
                nc.vector.copy_predicated(scd[:, 0:128], mask_sb[:], neg_sb[:])
                nc.vector.copy_predicated(scd[:, 512:640], mask_sb[:],
                                          neg_sb[:])
                psbd = p_p.tile([128, 2 * ST], BF16, tag="psb")
                nc.scalar.activation(psbd[:, 0:896], scd[:, 0:896], EXP,
                                     scale=0.125)
                nc.tensor.matmul(cxt[0:80, :], vp(j0), psbd[:, 0:512],
                                 start=first, stop=False)
                nc.tensor.matmul(cxt[0:80, 128:512], vp(j0 + 1),
                                 psbd[:, 512:896], start=False, stop=False)

                scd2 = sc_ps.tile([128, 2 * ST], F32, tag="sc")
                nc.tensor.matmul(scd2[:, 0:256], kt(j0 + 2), qh[:, 256:512],
                                 start=True, stop=True)
                nc.tensor.matmul(scd2[:, 256:384], kt(j0 + 3), qh[:, 384:512],
                                 start=True, stop=True)
                nc.vector.copy_predicated(scd2[:, 0:128], mask_sb[:],
                                          neg_sb[:])
                nc.vector.copy_predicated(scd2[:, 256:384], mask_sb[:],
                                          neg_sb[:])
                psbd2 = p_p.tile([128, 2 * ST], BF16, tag="psb")
                nc.scalar.activation(psbd2[:, 0:384], scd2[:, 0:384], EXP,
                                     scale=0.125)
                nc.tensor.matmul(cxt[0:80, 256:512], vp(j0 + 2),
                                 psbd2[:, 0:256], start=False, stop=False)
                nc.tensor.matmul(cxt[0:80, 384:512], vp(j0 + 3),
                                 psbd2[:, 256:384], start=False, stop=True)

                # ---- finalize: ctx.T -> [q, 80] via PE transpose ----
                cxs = cxs_p.tile([80, ST], BF16)
                nc.vector.tensor_copy(cxs[:], cxt[0:80, :])
                for qq in range(4):
                    fi = fi_ps.tile([128, 80], BF16, tag="fi")
                    nc.tensor.transpose(fi[:], cxs[:, qq * 128:(qq + 1) * 128],
                                        ident[0:80, 0:80])
                    rv = fo_p.tile([128, 1], F32, tag="rv")
                    nc.vector.reciprocal(rv[:], fi[:, 64:65])
                    nc.vector.tensor_scalar_mul(
                        ob[:, qq, h * 64:(h + 1) * 64], fi[:, 0:64], rv[:])

            nc.sync.dma_start(
                out=out_d[st * ST:(st + 1) * ST, :].rearrange(
                    "(q p) n -> p q n", p=128),
                in_=ob[:])
    return nc


_NC_CACHE = None


def _host_consts():
    i = np.arange(0, D, 2, dtype=np.float64) / D          # 32 pair exponents
    freqs = 1.0 / (10000.0 ** i)                           # (32,)
    ang = np.arange(S, dtype=np.float64)[:, None] * freqs[None, :]  # (S, 32)
    cos = np.cos(ang).astype(np.float32)                   # (S, 32)
    sin = np.sin(ang).astype(np.float32)
    dcol = (np.arange(RC) % D) // 2                        # (320,) pair idx
    cosn = np.ascontiguousarray(cos[:, dcol])              # (S, 320)
    sinn = np.ascontiguousarray(sin[:, dcol])
    kv, qq = np.meshgrid(np.arange(128), np.arange(128), indexing="ij")
    maskinv = (kv > qq).astype(np.uint8)                   # 1 = forbidden
    ident = np.eye(128, dtype=np.float32).astype(ml_dtypes.bfloat16)
    return cosn, sinn, maskinv, ident


def _in_maps(x, Wq, Wk, Wv):
    x = np.asarray(x, dtype=np.float32).reshape(B * S, DIN)
    xt = np.ascontiguousarray(x.T).astype(ml_dtypes.bfloat16)
    Wq = np.asarray(Wq, dtype=np.float32)
    Wk = np.asarray(Wk, dtype=np.float32)
    Wv = np.asarray(Wv, dtype=np.float32)
    cosn, sinn, maskinv, ident = _host_consts()

    in_maps = []
    for k in range(NCORES):
        w_all = np.hstack([
            Wq[:, k * 256:(k + 1) * 256],
            Wk[:, k * 64:(k + 1) * 64],
            Wv[:, k * 64:(k + 1) * 64],
        ]).astype(ml_dtypes.bfloat16)
        in_maps.append({
            "xt": xt, "w": np.ascontiguousarray(w_all),
            "cosn": cosn, "sinn": sinn, "mask": maskinv, "ident": ident,
        })
    return in_maps


def _run(in_maps, **kwargs):
    global _NC_CACHE
    if _NC_CACHE is None:
        _NC_CACHE = build_bass()
        _NC_CACHE.finalize()
    return run_bass_kernel_spmd(_NC_CACHE, in_maps, list(range(NCORES)),
                                **kwargs)


def kernel(x, Wq, Wk, Wv):
    res = _run(_in_maps(x, Wq, Wk, Wv))
    out = np.concatenate([res.results[k]["out"] for k in range(NCORES)], axis=1)
    return out.reshape(B, S, 32 * D)


# revision 15
# speedup vs baseline: 1.1852x; 1.1852x over previous
"""GQA forward (b=2, s=2048, H=32 q heads, 8 kv heads, d=64) on 8 TRN2 cores.

Sharding: core k owns query heads 4k..4k+3 and kv head k. GQA group
structure makes attention fully local per core (q heads 4k..4k+3 attend
only to kv head k). x is replicated; W columns are sharded; outputs are
column-concatenated.

v3b layout (all matmul operands bf16):
  - x transposed + bf16-cast on the HOST; x.T tiles DMA straight into SBUF.
  - Projections in TRANSPOSED layout: QKV.T[dout,s] blocks accumulated with
    W chunks stationary and x.T moving (512-wide streams, 48 matmuls/tile,
    and Q.T/K.T emerge directly -- no transposes needed for attention).
  - RoPE in transposed space via a host-side head-dim permutation (evens
    then odds): the pair rotation becomes a 32-row block swap, done with
    4 small SBUF->SBUF partition-offset DMA copies per q block, then
    3 DVE passes (cos-mul from PSUM, sin-mul, add).  Scores are invariant
    to the shared Q/K head-dim permutation; V is left unpermuted.
  - V.T rows flip back to natural [kv,d] via 4 DMA-transposes per s-tile
    straight into the [V|1|0pad] resident.
  - Attention in transposed layout: S.T[kv,q] = K @ Q.T per 128-kv block,
    two kv blocks share one PSUM tile so exp batches 2 strips per ACT
    instruction; causal via triangular predicated masks on diagonal
    blocks; ctx.T[80,q] = [V|1|0pad].T @ P.T accumulated in PSUM (row 64 =
    softmax sums, rows 65:80 zero pad).
  - Finalize: evict ctx.T to bf16 SBUF, PE-transpose back to [q,80],
    reciprocal+scale on DVE, assemble [128,4,256] f32, one DMA per s-tile.
"""

import numpy as np
from contextlib import ExitStack

import ml_dtypes

import concourse.bass as bass
import concourse.bacc as bacc
import concourse.mybir as mybir
from concourse import tile
from concourse.bass_utils import run_bass_kernel_spmd

F32 = mybir.dt.float32
BF16 = mybir.dt.bfloat16
U8 = mybir.dt.uint8
MUL = mybir.AluOpType.mult
ADD = mybir.AluOpType.add
EXP = mybir.ActivationFunctionType.Exp

B = 2
S = 2048
DIN = 2048
D = 64              # head dim
HPC = 4             # query heads per core
NCORES = 8
WCOLS = 4 * D + D + D  # 256 q cols + 64 k + 64 v = 384
ST = 512            # s-tile (rows per outer step)
NST = B * S // ST   # 8 s-tiles
NCH = DIN // 128    # 16 k-chunks
NKV = S // 128      # kv tiles per batch
NEG = -30000.0      # pre-scale mask fill; exp(NEG/8) == 0 in f32


def build_bass():
    nc = bacc.Bacc(None, target_bir_lowering=False)
    xt_d = nc.declare_dram_parameter("xt", [DIN, B * S], BF16, isOutput=False)
    w_d = nc.declare_dram_parameter("w", [DIN, WCOLS], BF16, isOutput=False)
    cos_d = nc.declare_dram_parameter("cost", [128, S], F32, isOutput=False)
    sin_d = nc.declare_dram_parameter("sint", [128, S], F32, isOutput=False)
    mask_d = nc.declare_dram_parameter("mask", [128, 128], U8, isOutput=False)
    id_d = nc.declare_dram_parameter("ident", [128, 128], BF16, isOutput=False)
    out_d = nc.declare_dram_parameter("out", [B * S, HPC * D], F32, isOutput=True)

    with ExitStack() as ctx:
        tc = ctx.enter_context(tile.TileContext(nc))
        const = ctx.enter_context(tc.tile_pool(name="const", bufs=1))
        resid = ctx.enter_context(tc.tile_pool(name="resid", bufs=1))
        xt_p = ctx.enter_context(tc.tile_pool(name="xt", bufs=3))
        qn_p = ctx.enter_context(tc.tile_pool(name="qn", bufs=3))
        qt_p = ctx.enter_context(tc.tile_pool(name="qt", bufs=2))
        p_p = ctx.enter_context(tc.tile_pool(name="p", bufs=4))
        cxs_p = ctx.enter_context(tc.tile_pool(name="cxs", bufs=3))
        fo_p = ctx.enter_context(tc.tile_pool(name="fo", bufs=3))
        ob_p = ctx.enter_context(tc.tile_pool(name="ob", bufs=2))
        pr_ps = ctx.enter_context(tc.tile_pool(name="pr_ps", bufs=2, space="PSUM"))
        sc_ps = ctx.enter_context(tc.tile_pool(name="sc_ps", bufs=2, space="PSUM"))
        cx_ps = ctx.enter_context(tc.tile_pool(name="cx_ps", bufs=1, space="PSUM"))
        fi_ps = ctx.enter_context(tc.tile_pool(name="fi_ps", bufs=1, space="PSUM"))

        # constants / residents
        w_sb = const.tile([128, NCH, WCOLS], BF16)
        nc.sync.dma_start(
            out=w_sb[:], in_=w_d.rearrange("(c p) n -> p c n", p=128))
        mask_sb = const.tile([128, 128], U8)
        nc.sync.dma_start(out=mask_sb[:], in_=mask_d[:])
        ident = const.tile([128, 128], BF16)
        nc.sync.dma_start(out=ident[:], in_=id_d[:])
        cos_sb = const.tile([128, S], F32)
        nc.sync.dma_start(out=cos_sb[:], in_=cos_d[:])
        sin_sb = const.tile([128, S], F32)
        nc.sync.dma_start(out=sin_sb[:], in_=sin_d[:])
        neg_sb = const.tile([128, 128], F32)
        nc.vector.memset(neg_sb[:], NEG)

        # rows 0-63: K.T (RoPE'd); rows 64-127: duplicate copy so the scores
        # matmul lhsT can match either base partition of the Q halves
        kt_res = resid.tile([128, B * S], BF16)
        # [V | 1 | 0-pad] per kv tile: col 64 = ones (softmax sums land in
        # ctx.T row 64), cols 65:80 zero so ctx.T rows 65:80 read initialized
        vp_res = resid.tile([128, B * NKV, 80], BF16)
        nc.vector.memset(vp_res[:, :, 64:65], 1.0)
        nc.vector.memset(vp_res[:, :, 65:80], 0.0)

        xt_view = xt_d.rearrange("(c p) s -> p c s", p=128)

        for st in range(NST):
            b, sti = divmod(st, 4)
            ssl = slice(st * ST, (st + 1) * ST)        # kt_res col range
            tsl = slice(sti * ST, (sti + 1) * ST)      # within-batch cols

            # ---- x.T tile straight from DRAM (host pre-transposed) ----
            xt = xt_p.tile([128, NCH, ST], BF16)
            nc.sync.dma_start(out=xt[:], in_=xt_view[:, :, ssl])

            # ---- transposed projections + rotate-half RoPE ----
            qta = qt_p.tile([128, ST], BF16, tag="qta")   # heads 0,1 as [d,s]
            qtb = qt_p.tile([128, ST], BF16, tag="qtb")   # heads 2,3 as [d,s]
            for blk in range(3):
                pp = pr_ps.tile([128, ST], F32, tag="pp")
                for c in range(NCH):
                    nc.tensor.matmul(
                        pp[:], w_sb[:, c, blk * 128:(blk + 1) * 128],
                        xt[:, c, :], start=(c == 0), stop=(c == NCH - 1))
                raw = qn_p.tile([128, ST], BF16, tag="raw")
                nc.vector.tensor_copy(raw[:], pp[:])
                swp = qn_p.tile([128, ST], BF16, tag="swp")
                ngrp = 2 if blk < 2 else 1
                for g in range(ngrp):
                    nc.sync.dma_start(out=swp[g * 64:g * 64 + 32, :],
                                      in_=raw[g * 64 + 32:g * 64 + 64, :])
                    nc.sync.dma_start(out=swp[g * 64 + 32:g * 64 + 64, :],
                                      in_=raw[g * 64:g * 64 + 32, :])
                ts = qn_p.tile([128, ST], BF16, tag="ts")
                if blk < 2:
                    dst = (qta if blk == 0 else qtb)
                    nc.vector.tensor_tensor(dst[:], pp[:], cos_sb[:, tsl], MUL)
                    nc.vector.tensor_tensor(ts[:], swp[:], sin_sb[:, tsl], MUL)
                    nc.vector.tensor_tensor(dst[:], dst[:], ts[:], ADD)
                else:
                    kd = kt_res[0:64, ssl]
                    nc.vector.tensor_tensor(kd, pp[0:64, :],
                                            cos_sb[0:64, tsl], MUL)
                    nc.vector.tensor_tensor(ts[0:64, :], swp[0:64, :],
                                            sin_sb[0:64, tsl], MUL)
                    nc.vector.tensor_tensor(kd, kd, ts[0:64, :], ADD)
                    # V.T rows -> natural [kv, d] into the resident
                    for ptb in range(4):
                        nc.sync.dma_start(
                            out=vp_res[:, b * NKV + sti * 4 + ptb, 0:64],
                            in_=raw[64:128, ptb * 128:(ptb + 1) * 128],
                            transpose=True)
            nc.sync.dma_start(out=kt_res[64:128, ssl], in_=kt_res[0:64, ssl])

            ob = ob_p.tile([128, 4, HPC * D], F32)

            # ---- attention for the 4 heads of this q-tile ----
            for h in range(HPC):
                p0 = (h % 2) * 64
                qh = (qta if h < 2 else qtb)[p0:p0 + 64, :]

                def kt(j):
                    return kt_res[p0:p0 + 64,
                                  b * S + j * 128:b * S + (j + 1) * 128]

                def vp(j):
                    return vp_res[:, b * NKV + j, :]

                cxt = cx_ps.tile([128, ST], F32, tag="cxt")
                first = True
                # full sub-diagonal blocks, two kv blocks per PSUM tile
                for jp in range(0, 4 * sti, 2):
                    sc = sc_ps.tile([128, 2, ST], F32, tag="sc")
                    for jj in (0, 1):
                        nc.tensor.matmul(sc[:, jj, :], kt(jp + jj), qh[:, :],
                                         start=True, stop=True)
                    psb = p_p.tile([128, 2, ST], BF16, tag="psb")
                    nc.scalar.activation(psb[:], sc[:], EXP, scale=0.125)
                    for jj in (0, 1):
                        nc.tensor.matmul(
                            cxt[0:80, :], vp(jp + jj), psb[:, jj, :],
                            start=first, stop=False)
                        first = False
                # diagonal strips r=0..3 (kv block 4*sti+r vs q cols
                # 128r:512), packed two per PSUM tile
                j0 = 4 * sti
                scd = sc_ps.tile([128, 2 * ST], F32, tag="sc")
                nc.tensor.matmul(scd[:, 0:512], kt(j0), qh[:, :],
                                 start=True, stop=True)
                nc.tensor.matmul(scd[:, 512:896], kt(j0 + 1), qh[:, 128:512],
                                 start=True, stop=True)
                nc.vector.copy_predicated(scd[:, 0:128], mask_sb[:], neg_sb[:])
                nc.vector.copy_predicated(scd[:, 512:640], mask_sb[:],
                                          neg_sb[:])
                psbd = p_p.tile([128, 2 * ST], BF16, tag="psb")
                nc.scalar.activation(psbd[:, 0:896], scd[:, 0:896], EXP,
                                     scale=0.125)
                nc.tensor.matmul(cxt[0:80, :], vp(j0), psbd[:, 0:512],
                                 start=first, stop=False)
                nc.tensor.matmul(cxt[0:80, 128:512], vp(j0 + 1),
                                 psbd[:, 512:896], start=False, stop=False)

                scd2 = sc_ps.tile([128, 2 * ST], F32, tag="sc")
                nc.tensor.matmul(scd2[:, 0:256], kt(j0 + 2), qh[:, 256:512],
                                 start=True, stop=True)
                nc.tensor.matmul(scd2[:, 256:384], kt(j0 + 3), qh[:, 384:512],
                                 start=True, stop=True)
                nc.vector.copy_predicated(scd2[:, 0:128], mask_sb[:],
                                          neg_sb[:])
                nc.vector.copy_predicated(scd2[:, 256:384], mask_sb[:],
                                          neg_sb[:])
                psbd2 = p_p.tile([128, 2 * ST], BF16, tag="psb")
                nc.scalar.activation(psbd2[:, 0:384], scd2[:, 0:384], EXP,
                                     scale=0.125)
                nc.tensor.matmul(cxt[0:80, 256:512], vp(j0 + 2),
                                 psbd2[:, 0:256], start=False, stop=False)
                nc.tensor.matmul(cxt[0:80, 384:512], vp(j0 + 3),
                                 psbd2[:, 256:384], start=False, stop=True)

                # ---- finalize: ctx.T -> [q, 80] via PE transpose ----
                cxs = cxs_p.tile([80, ST], BF16)
                nc.vector.tensor_copy(cxs[:], cxt[0:80, :])
                for qq in range(4):
                    fi = fi_ps.tile([128, 80], BF16, tag="fi")
                    nc.tensor.transpose(fi[:], cxs[:, qq * 128:(qq + 1) * 128],
                                        ident[0:80, 0:80])
                    rv = fo_p.tile([128, 1], F32, tag="rv")
                    nc.vector.reciprocal(rv[:], fi[:, 64:65])
                    nc.vector.tensor_scalar_mul(
                        ob[:, qq, h * 64:(h + 1) * 64], fi[:, 0:64], rv[:])

            nc.sync.dma_start(
                out=out_d[st * ST:(st + 1) * ST, :].rearrange(
                    "(q p) n -> p q n", p=128),
                in_=ob[:])
    return nc


_NC_CACHE = None

# head-dim permutation shared by Q and K: evens then odds (scores are
# invariant; turns interleaved-pair RoPE into a 32-row rotate-half)
_PERM64 = np.concatenate([np.arange(0, D, 2), np.arange(1, D, 2)])


def _host_consts():
    freqs = 1.0 / (10000.0 ** (np.arange(32, dtype=np.float64) * 2 / D))
    ang = freqs[:, None] * np.arange(S, dtype=np.float64)[None, :]  # (32, S)
    cos32 = np.cos(ang)
    sin32 = np.sin(ang)
    rr = np.arange(128) % D
    p32 = rr % 32
    cosT = cos32[p32, :].astype(np.float32)                    # (128, S)
    sgn = np.where(rr < 32, -1.0, 1.0)[:, None]
    sinT = (sin32[p32, :] * sgn).astype(np.float32)            # (128, S)
    kv, qq = np.meshgrid(np.arange(128), np.arange(128), indexing="ij")
    maskinv = (kv > qq).astype(np.uint8)                       # 1 = forbidden
    ident = np.eye(128, dtype=np.float32).astype(ml_dtypes.bfloat16)
    return cosT, sinT, maskinv, ident


def _in_maps(x, Wq, Wk, Wv):
    x = np.asarray(x, dtype=np.float32).reshape(B * S, DIN)
    xt = np.ascontiguousarray(x.T).astype(ml_dtypes.bfloat16)
    Wq = np.asarray(Wq, dtype=np.float32)
    Wk = np.asarray(Wk, dtype=np.float32)
    Wv = np.asarray(Wv, dtype=np.float32)
    # permute head-dims of Q and K weights (evens first) for rotate-half rope
    Wq = Wq.reshape(DIN, 32, D)[:, :, _PERM64].reshape(DIN, 32 * D)
    Wk = Wk.reshape(DIN, 8, D)[:, :, _PERM64].reshape(DIN, 8 * D)
    cosT, sinT, maskinv, ident = _host_consts()

    in_maps = []
    for k in range(NCORES):
        w_all = np.hstack([
            Wq[:, k * 256:(k + 1) * 256],
            Wk[:, k * 64:(k + 1) * 64],
            Wv[:, k * 64:(k + 1) * 64],
        ]).astype(ml_dtypes.bfloat16)
        in_maps.append({
            "xt": xt, "w": np.ascontiguousarray(w_all),
            "cost": cosT, "sint": sinT, "mask": maskinv, "ident": ident,
        })
    return in_maps


def _run(in_maps, **kwargs):
    global _NC_CACHE
    if _NC_CACHE is None:
        _NC_CACHE = build_bass()
        _NC_CACHE.finalize()
    return run_bass_kernel_spmd(_NC_CACHE, in_maps, list(range(NCORES)),
                                **kwargs)


def kernel(x, Wq, Wk, Wv):
    res = _run(_in_maps(x, Wq, Wk, Wv))
    out = np.concatenate([res.results[k]["out"] for k in range(NCORES)], axis=1)
    return out.reshape(B, S, 32 * D)


# revision 21
# speedup vs baseline: 1.3098x; 1.1052x over previous
"""GQA forward (b=2, s=2048, H=32 q heads, 8 kv heads, d=64) on 8 TRN2 cores.

Sharding: core k owns query heads 4k..4k+3 and kv head k. GQA group
structure makes attention fully local per core (q heads 4k..4k+3 attend
only to kv head k). x is replicated; W columns are sharded; outputs are
column-concatenated.

v3b layout (all matmul operands bf16):
  - x transposed + bf16-cast on the HOST; x.T tiles DMA straight into SBUF.
  - Projections in TRANSPOSED layout: QKV.T[dout,s] blocks accumulated with
    W chunks stationary and x.T moving (512-wide streams, 48 matmuls/tile,
    and Q.T/K.T emerge directly -- no transposes needed for attention).
  - RoPE in transposed space via a host-side head-dim permutation (evens
    then odds): the pair rotation becomes a 32-row block swap, done with
    4 small SBUF->SBUF partition-offset DMA copies per q block, then
    3 DVE passes (cos-mul from PSUM, sin-mul, add).  Scores are invariant
    to the shared Q/K head-dim permutation; V is left unpermuted.
  - V.T rows flip back to natural [kv,d] via 4 DMA-transposes per s-tile
    straight into the [V|1|0pad] resident.
  - Attention in transposed layout: S.T[kv,q] = K @ Q.T per 128-kv block,
    two kv blocks share one PSUM tile so exp batches 2 strips per ACT
    instruction; causal via triangular predicated masks on diagonal
    blocks; ctx.T[80,q] = [V|1|0pad].T @ P.T accumulated in PSUM (row 64 =
    softmax sums, rows 65:80 zero pad).
  - Finalize: evict ctx.T to bf16 SBUF, PE-transpose back to [q,80],
    reciprocal+scale on DVE, assemble [128,4,256] f32, one DMA per s-tile.
"""

import numpy as np
from contextlib import ExitStack

import ml_dtypes

import concourse.bass as bass
import concourse.bacc as bacc
import concourse.mybir as mybir
from concourse import tile
from concourse.bass_utils import run_bass_kernel_spmd

F32 = mybir.dt.float32
BF16 = mybir.dt.bfloat16
U8 = mybir.dt.uint8
MUL = mybir.AluOpType.mult
ADD = mybir.AluOpType.add
EXP = mybir.ActivationFunctionType.Exp

B = 2
S = 2048
DIN = 2048
D = 64              # head dim
HPC = 4             # query heads per core
NCORES = 8
WCOLS = 4 * D + D + D  # 256 q cols + 64 k + 64 v = 384
ST = 512            # s-tile (rows per outer step)
NST = B * S // ST   # 8 s-tiles
NCH = DIN // 128    # 16 k-chunks
NKV = S // 128      # kv tiles per batch
NEG = -30000.0      # pre-scale mask fill; exp(NEG/8) == 0 in f32


def build_bass():
    nc = bacc.Bacc(None, target_bir_lowering=False)
    xt_d = nc.declare_dram_parameter("xt", [DIN, B * S], BF16, isOutput=False)
    w_d = nc.declare_dram_parameter("w", [DIN, WCOLS], BF16, isOutput=False)
    cos_d = nc.declare_dram_parameter("cost", [128, S], F32, isOutput=False)
    sin_d = nc.declare_dram_parameter("sint", [128, S], F32, isOutput=False)
    mask_d = nc.declare_dram_parameter("mask", [128, 128], U8, isOutput=False)
    id_d = nc.declare_dram_parameter("ident", [128, 128], BF16, isOutput=False)
    out_d = nc.declare_dram_parameter("out", [B * S, HPC * D], F32, isOutput=True)

    with ExitStack() as ctx:
        tc = ctx.enter_context(tile.TileContext(nc))
        const = ctx.enter_context(tc.tile_pool(name="const", bufs=1))
        resid = ctx.enter_context(tc.tile_pool(name="resid", bufs=1))
        xt_p = ctx.enter_context(tc.tile_pool(name="xt", bufs=3))
        qn_p = ctx.enter_context(tc.tile_pool(name="qn", bufs=3))
        qt_p = ctx.enter_context(tc.tile_pool(name="qt", bufs=2))
        p_p = ctx.enter_context(tc.tile_pool(name="p", bufs=4))
        cxs_p = ctx.enter_context(tc.tile_pool(name="cxs", bufs=3))
        fo_p = ctx.enter_context(tc.tile_pool(name="fo", bufs=3))
        ob_p = ctx.enter_context(tc.tile_pool(name="ob", bufs=3))
        pr_ps = ctx.enter_context(tc.tile_pool(name="pr_ps", bufs=2, space="PSUM"))
        sc_ps = ctx.enter_context(tc.tile_pool(name="sc_ps", bufs=2, space="PSUM"))
        cx_ps = ctx.enter_context(tc.tile_pool(name="cx_ps", bufs=1, space="PSUM"))
        fi_ps = ctx.enter_context(tc.tile_pool(name="fi_ps", bufs=1, space="PSUM"))

        # constants / residents
        w_sb = const.tile([128, NCH, WCOLS], BF16)
        nc.sync.dma_start(
            out=w_sb[:], in_=w_d.rearrange("(c p) n -> p c n", p=128))
        mask_sb = const.tile([128, 128], U8)
        nc.sync.dma_start(out=mask_sb[:], in_=mask_d[:])
        ident = const.tile([128, 128], BF16)
        nc.sync.dma_start(out=ident[:], in_=id_d[:])
        cos_sb = const.tile([128, S], F32)
        nc.sync.dma_start(out=cos_sb[:], in_=cos_d[:])
        sin_sb = const.tile([128, S], F32)
        nc.sync.dma_start(out=sin_sb[:], in_=sin_d[:])
        neg_sb = const.tile([128, 128], F32)
        nc.vector.memset(neg_sb[:], NEG)

        # Two K.T residents, zero-padded to k=128 so the scores matmul always
        # contracts over the full partition range with the full Q pair tile:
        # kt_res = [K.T; 0] selects the even head of a pair, kt2_res = [0; K.T]
        # the odd head (the zero half annihilates the other head's Q rows).
        kt_res = resid.tile([128, B * S], BF16)
        kt2_res = resid.tile([128, B * S], BF16)
        nc.vector.memset(kt_res[64:128, :], 0.0)
        nc.vector.memset(kt2_res[0:64, :], 0.0)
        # [V | 1 | 0-pad] per kv tile: col 64 = ones (softmax sums land in
        # ctx.T row 64), cols 65:80 zero so ctx.T rows 65:80 read initialized
        vp_res = resid.tile([128, B * NKV, 80], BF16)
        nc.vector.memset(vp_res[:, :, 64:65], 1.0)
        nc.vector.memset(vp_res[:, :, 65:80], 0.0)

        xt_view = xt_d.rearrange("(c p) s -> p c s", p=128)

        for st in range(NST):
            b, sti = divmod(st, 4)
            ssl = slice(st * ST, (st + 1) * ST)        # kt_res col range
            tsl = slice(sti * ST, (sti + 1) * ST)      # within-batch cols

            # ---- x.T tile straight from DRAM (host pre-transposed);
            # split so the first projection chunk can start early ----
            xt = xt_p.tile([128, NCH, ST], BF16)
            for cc in range(0, NCH, 4):
                nc.sync.dma_start(out=xt[:, cc:cc + 4, :],
                                  in_=xt_view[:, cc:cc + 4, ssl])

            # ---- transposed projections + rotate-half RoPE ----
            qta = qt_p.tile([128, ST], BF16, tag="qta")   # heads 0,1 as [d,s]
            qtb = qt_p.tile([128, ST], BF16, tag="qtb")   # heads 2,3 as [d,s]
            for blk in range(3):
                pp = pr_ps.tile([128, ST], F32, tag="pp")
                for c in range(NCH):
                    nc.tensor.matmul(
                        pp[:], w_sb[:, c, blk * 128:(blk + 1) * 128],
                        xt[:, c, :], start=(c == 0), stop=(c == NCH - 1))
                raw = qn_p.tile([128, ST], BF16, tag="raw")
                nc.vector.tensor_copy(raw[:], pp[:])
                swp = qn_p.tile([128, ST], BF16, tag="swp")
                ngrp = 2 if blk < 2 else 1
                for g in range(ngrp):
                    nc.sync.dma_start(out=swp[g * 64:g * 64 + 32, :],
                                      in_=raw[g * 64 + 32:g * 64 + 64, :])
                    nc.sync.dma_start(out=swp[g * 64 + 32:g * 64 + 64, :],
                                      in_=raw[g * 64:g * 64 + 32, :])
                ts = qn_p.tile([128, ST], BF16, tag="ts")
                if blk < 2:
                    dst = (qta if blk == 0 else qtb)
                    nc.vector.tensor_tensor(dst[:], pp[:], cos_sb[:, tsl], MUL)
                    nc.vector.tensor_tensor(ts[:], swp[:], sin_sb[:, tsl], MUL)
                    nc.vector.tensor_tensor(dst[:], dst[:], ts[:], ADD)
                else:
                    kd = kt_res[0:64, ssl]
                    nc.vector.tensor_tensor(kd, pp[0:64, :],
                                            cos_sb[0:64, tsl], MUL)
                    nc.vector.tensor_tensor(ts[0:64, :], swp[0:64, :],
                                            sin_sb[0:64, tsl], MUL)
                    nc.vector.tensor_tensor(kd, kd, ts[0:64, :], ADD)
                    # V.T rows -> natural [kv, d] into the resident
                    for ptb in range(4):
                        nc.sync.dma_start(
                            out=vp_res[:, b * NKV + sti * 4 + ptb, 0:64],
                            in_=raw[64:128, ptb * 128:(ptb + 1) * 128],
                            transpose=True)
            nc.sync.dma_start(out=kt2_res[64:128, ssl], in_=kt_res[0:64, ssl])

            ob = ob_p.tile([128, 4, HPC * D], F32)

            # ---- attention for the 4 heads of this q-tile ----
            for h in range(HPC):
                qh = (qta if h < 2 else qtb)[:, :]
                ktsrc = kt_res if h % 2 == 0 else kt2_res

                def kt(j):
                    return ktsrc[:, b * S + j * 128:b * S + (j + 1) * 128]

                def vp(j):
                    return vp_res[:, b * NKV + j, :]

                cxt = cx_ps.tile([128, ST], F32, tag="cxt")
                first = True
                # full sub-diagonal blocks, two kv blocks per PSUM tile
                for jp in range(0, 4 * sti, 2):
                    sc = sc_ps.tile([128, 2, ST], F32, tag="sc")
                    for jj in (0, 1):
                        nc.tensor.matmul(sc[:, jj, :], kt(jp + jj), qh[:, :],
                                         start=True, stop=True)
                    psb = p_p.tile([128, 2, ST], BF16, tag="psb")
                    nc.scalar.activation(psb[:], sc[:], EXP, scale=0.125)
                    for jj in (0, 1):
                        nc.tensor.matmul(
                            cxt[0:80, :], vp(jp + jj), psb[:, jj, :],
                            start=first, stop=False)
                        first = False
                # diagonal strips r=0..3 (kv block 4*sti+r vs q cols
                # 128r:512), packed two per PSUM tile
                j0 = 4 * sti
                scd = sc_ps.tile([128, 2 * ST], F32, tag="sc")
                nc.tensor.matmul(scd[:, 0:512], kt(j0), qh[:, :],
                                 start=True, stop=True)
                nc.tensor.matmul(scd[:, 512:896], kt(j0 + 1), qh[:, 128:512],
                                 start=True, stop=True)
                nc.vector.copy_predicated(scd[:, 0:128], mask_sb[:], neg_sb[:])
                nc.vector.copy_predicated(scd[:, 512:640], mask_sb[:],
                                          neg_sb[:])
                psbd = p_p.tile([128, 2 * ST], BF16, tag="psb")
                nc.scalar.activation(psbd[:, 0:896], scd[:, 0:896], EXP,
                                     scale=0.125)
                nc.tensor.matmul(cxt[0:80, :], vp(j0), psbd[:, 0:512],
                                 start=first, stop=False)
                nc.tensor.matmul(cxt[0:80, 128:512], vp(j0 + 1),
                                 psbd[:, 512:896], start=False, stop=False)

                scd2 = sc_ps.tile([128, 2 * ST], F32, tag="sc")
                nc.tensor.matmul(scd2[:, 0:256], kt(j0 + 2), qh[:, 256:512],
                                 start=True, stop=True)
                nc.tensor.matmul(scd2[:, 256:384], kt(j0 + 3), qh[:, 384:512],
                                 start=True, stop=True)
                nc.vector.copy_predicated(scd2[:, 0:128], mask_sb[:],
                                          neg_sb[:])
                nc.vector.copy_predicated(scd2[:, 256:384], mask_sb[:],
                                          neg_sb[:])
                psbd2 = p_p.tile([128, 2 * ST], BF16, tag="psb")
                nc.scalar.activation(psbd2[:, 0:384], scd2[:, 0:384], EXP,
                                     scale=0.125)
                nc.tensor.matmul(cxt[0:80, 256:512], vp(j0 + 2),
                                 psbd2[:, 0:256], start=False, stop=False)
                nc.tensor.matmul(cxt[0:80, 384:512], vp(j0 + 3),
                                 psbd2[:, 256:384], start=False, stop=True)

                # ---- finalize: ctx.T -> [q, 80] via PE transpose ----
                # (eviction on ACT: Copy shares the Exp activation table)
                cxs = cxs_p.tile([80, ST], BF16)
                nc.scalar.activation(cxs[:], cxt[0:80, :],
                                     mybir.ActivationFunctionType.Copy)
                for qq in range(4):
                    fi = fi_ps.tile([128, 80], BF16, tag="fi")
                    nc.tensor.transpose(fi[:], cxs[:, qq * 128:(qq + 1) * 128],
                                        ident[0:80, 0:80])
                    rv = fo_p.tile([128, 1], F32, tag="rv")
                    nc.vector.reciprocal(rv[:], fi[:, 64:65])
                    nc.vector.tensor_scalar_mul(
                        ob[:, qq, h * 64:(h + 1) * 64], fi[:, 0:64], rv[:])

            nc.sync.dma_start(
                out=out_d[st * ST:(st + 1) * ST, :].rearrange(
                    "(q p) n -> p q n", p=128),
                in_=ob[:])
    return nc


_NC_CACHE = None

# head-dim permutation shared by Q and K: evens then odds (scores are
# invariant; turns interleaved-pair RoPE into a 32-row rotate-half)
_PERM64 = np.concatenate([np.arange(0, D, 2), np.arange(1, D, 2)])


def _host_consts():
    freqs = 1.0 / (10000.0 ** (np.arange(32, dtype=np.float64) * 2 / D))
    ang = freqs[:, None] * np.arange(S, dtype=np.float64)[None, :]  # (32, S)
    cos32 = np.cos(ang)
    sin32 = np.sin(ang)
    rr = np.arange(128) % D
    p32 = rr % 32
    cosT = cos32[p32, :].astype(np.float32)                    # (128, S)
    sgn = np.where(rr < 32, -1.0, 1.0)[:, None]
    sinT = (sin32[p32, :] * sgn).astype(np.float32)            # (128, S)
    kv, qq = np.meshgrid(np.arange(128), np.arange(128), indexing="ij")
    maskinv = (kv > qq).astype(np.uint8)                       # 1 = forbidden
    ident = np.eye(128, dtype=np.float32).astype(ml_dtypes.bfloat16)
    return cosT, sinT, maskinv, ident


def _in_maps(x, Wq, Wk, Wv):
    x = np.asarray(x, dtype=np.float32).reshape(B * S, DIN)
    xt = np.ascontiguousarray(x.T).astype(ml_dtypes.bfloat16)
    Wq = np.asarray(Wq, dtype=np.float32)
    Wk = np.asarray(Wk, dtype=np.float32)
    Wv = np.asarray(Wv, dtype=np.float32)
    # permute head-dims of Q and K weights (evens first) for rotate-half rope
    Wq = Wq.reshape(DIN, 32, D)[:, :, _PERM64].reshape(DIN, 32 * D)
    Wk = Wk.reshape(DIN, 8, D)[:, :, _PERM64].reshape(DIN, 8 * D)
    cosT, sinT, maskinv, ident = _host_consts()

    in_maps = []
    for k in range(NCORES):
        w_all = np.hstack([
            Wq[:, k * 256:(k + 1) * 256],
            Wk[:, k * 64:(k + 1) * 64],
            Wv[:, k * 64:(k + 1) * 64],
        ]).astype(ml_dtypes.bfloat16)
        in_maps.append({
            "xt": xt, "w": np.ascontiguousarray(w_all),
            "cost": cosT, "sint": sinT, "mask": maskinv, "ident": ident,
        })
    return in_maps


def _run(in_maps, **kwargs):
    global _NC_CACHE
    if _NC_CACHE is None:
        _NC_CACHE = build_bass()
        _NC_CACHE.finalize()
    return run_bass_kernel_spmd(_NC_CACHE, in_maps, list(range(NCORES)),
                                **kwargs)


def kernel(x, Wq, Wk, Wv):
    res = _run(_in_maps(x, Wq, Wk, Wv))
    out = np.concatenate([res.results[k]["out"] for k in range(NCORES)], axis=1)
    return out.reshape(B, S, 32 * D)


# revision 23
# speedup vs baseline: 1.3193x; 1.0072x over previous
"""GQA forward (b=2, s=2048, H=32 q heads, 8 kv heads, d=64) on 8 TRN2 cores.

Sharding: core k owns query heads 4k..4k+3 and kv head k. GQA group
structure makes attention fully local per core (q heads 4k..4k+3 attend
only to kv head k). x is replicated; W columns are sharded; outputs are
column-concatenated.

v3b layout (all matmul operands bf16):
  - x transposed + bf16-cast on the HOST; x.T tiles DMA straight into SBUF.
  - Projections in TRANSPOSED layout: QKV.T[dout,s] blocks accumulated with
    W chunks stationary and x.T moving (512-wide streams, 48 matmuls/tile,
    and Q.T/K.T emerge directly -- no transposes needed for attention).
  - RoPE in transposed space via a host-side head-dim permutation (evens
    then odds): the pair rotation becomes a 32-row block swap, done with
    4 small SBUF->SBUF partition-offset DMA copies per q block, then
    3 DVE passes (cos-mul from PSUM, sin-mul, add).  Scores are invariant
    to the shared Q/K head-dim permutation; V is left unpermuted.
  - V.T rows flip back to natural [kv,d] via 4 DMA-transposes per s-tile
    straight into the [V|1|0pad] resident.
  - Attention in transposed layout: S.T[kv,q] = K @ Q.T per 128-kv block,
    two kv blocks share one PSUM tile so exp batches 2 strips per ACT
    instruction; causal via triangular predicated masks on diagonal
    blocks; ctx.T[80,q] = [V|1|0pad].T @ P.T accumulated in PSUM (row 64 =
    softmax sums, rows 65:80 zero pad).
  - Finalize: evict ctx.T to bf16 SBUF, PE-transpose back to [q,80],
    reciprocal+scale on DVE, assemble [128,4,256] f32, one DMA per s-tile.
"""

import numpy as np
from contextlib import ExitStack

import ml_dtypes

import concourse.bass as bass
import concourse.bacc as bacc
import concourse.mybir as mybir
from concourse import tile
from concourse.bass_utils import run_bass_kernel_spmd

F32 = mybir.dt.float32
BF16 = mybir.dt.bfloat16
U8 = mybir.dt.uint8
MUL = mybir.AluOpType.mult
ADD = mybir.AluOpType.add
EXP = mybir.ActivationFunctionType.Exp

B = 2
S = 2048
DIN = 2048
D = 64              # head dim
HPC = 4             # query heads per core
NCORES = 8
WCOLS = 4 * D + D + D  # 256 q cols + 64 k + 64 v = 384
ST = 512            # s-tile (rows per outer step)
NST = B * S // ST   # 8 s-tiles
NCH = DIN // 128    # 16 k-chunks
NKV = S // 128      # kv tiles per batch
NEG = -30000.0      # pre-scale mask fill; exp(NEG/8) == 0 in f32


def build_bass():
    nc = bacc.Bacc(None, target_bir_lowering=False)
    xt_d = nc.declare_dram_parameter("xt", [DIN, B * S], BF16, isOutput=False)
    w_d = nc.declare_dram_parameter("w", [DIN, WCOLS], BF16, isOutput=False)
    cos_d = nc.declare_dram_parameter("cost", [128, S], F32, isOutput=False)
    sin_d = nc.declare_dram_parameter("sint", [128, S], F32, isOutput=False)
    mask_d = nc.declare_dram_parameter("mask", [128, 128], U8, isOutput=False)
    id_d = nc.declare_dram_parameter("ident", [128, 128], BF16, isOutput=False)
    out_d = nc.declare_dram_parameter("out", [B * S, HPC * D], F32, isOutput=True)

    with ExitStack() as ctx:
        tc = ctx.enter_context(tile.TileContext(nc))
        const = ctx.enter_context(tc.tile_pool(name="const", bufs=1))
        resid = ctx.enter_context(tc.tile_pool(name="resid", bufs=1))
        xt_p = ctx.enter_context(tc.tile_pool(name="xt", bufs=3))
        qn_p = ctx.enter_context(tc.tile_pool(name="qn", bufs=3))
        qt_p = ctx.enter_context(tc.tile_pool(name="qt", bufs=2))
        p_p = ctx.enter_context(tc.tile_pool(name="p", bufs=6))
        cxs_p = ctx.enter_context(tc.tile_pool(name="cxs", bufs=3))
        fo_p = ctx.enter_context(tc.tile_pool(name="fo", bufs=3))
        ob_p = ctx.enter_context(tc.tile_pool(name="ob", bufs=3))
        pr_ps = ctx.enter_context(tc.tile_pool(name="pr_ps", bufs=2, space="PSUM"))
        sc_ps = ctx.enter_context(tc.tile_pool(name="sc_ps", bufs=2, space="PSUM"))
        cx_ps = ctx.enter_context(tc.tile_pool(name="cx_ps", bufs=1, space="PSUM"))
        fi_ps = ctx.enter_context(tc.tile_pool(name="fi_ps", bufs=1, space="PSUM"))

        # constants / residents
        w_sb = const.tile([128, NCH, WCOLS], BF16)
        nc.sync.dma_start(
            out=w_sb[:], in_=w_d.rearrange("(c p) n -> p c n", p=128))
        mask_sb = const.tile([128, 128], U8)
        nc.sync.dma_start(out=mask_sb[:], in_=mask_d[:])
        ident = const.tile([128, 128], BF16)
        nc.sync.dma_start(out=ident[:], in_=id_d[:])
        cos_sb = const.tile([128, S], F32)
        nc.sync.dma_start(out=cos_sb[:], in_=cos_d[:])
        sin_sb = const.tile([128, S], F32)
        nc.sync.dma_start(out=sin_sb[:], in_=sin_d[:])
        neg_sb = const.tile([128, 128], F32)
        nc.vector.memset(neg_sb[:], NEG)

        # Two K.T residents, zero-padded to k=128 so the scores matmul always
        # contracts over the full partition range with the full Q pair tile:
        # kt_res = [K.T; 0] selects the even head of a pair, kt2_res = [0; K.T]
        # the odd head (the zero half annihilates the other head's Q rows).
        kt_res = resid.tile([128, B * S], BF16)
        kt2_res = resid.tile([128, B * S], BF16)
        nc.vector.memset(kt_res[64:128, :], 0.0)
        nc.vector.memset(kt2_res[0:64, :], 0.0)
        # [V | 1 | 0-pad] per kv tile: col 64 = ones (softmax sums land in
        # ctx.T row 64), cols 65:80 zero so ctx.T rows 65:80 read initialized
        vp_res = resid.tile([128, B * NKV, 80], BF16)
        nc.vector.memset(vp_res[:, :, 64:65], 1.0)
        nc.vector.memset(vp_res[:, :, 65:80], 0.0)

        xt_view = xt_d.rearrange("(c p) s -> p c s", p=128)

        for st in range(NST):
            b, sti = divmod(st, 4)
            ssl = slice(st * ST, (st + 1) * ST)        # kt_res col range
            tsl = slice(sti * ST, (sti + 1) * ST)      # within-batch cols

            # ---- x.T tile straight from DRAM (host pre-transposed);
            # split so the first projection chunk can start early ----
            xt = xt_p.tile([128, NCH, ST], BF16)
            for cc in range(0, NCH, 4):
                nc.sync.dma_start(out=xt[:, cc:cc + 4, :],
                                  in_=xt_view[:, cc:cc + 4, ssl])

            # ---- transposed projections + rotate-half RoPE ----
            qta = qt_p.tile([128, ST], BF16, tag="qta")   # heads 0,1 as [d,s]
            qtb = qt_p.tile([128, ST], BF16, tag="qtb")   # heads 2,3 as [d,s]
            for blk in range(3):
                pp = pr_ps.tile([128, ST], F32, tag="pp")
                for c in range(NCH):
                    nc.tensor.matmul(
                        pp[:], w_sb[:, c, blk * 128:(blk + 1) * 128],
                        xt[:, c, :], start=(c == 0), stop=(c == NCH - 1))
                raw = qn_p.tile([128, ST], BF16, tag="raw")
                nc.vector.tensor_copy(raw[:], pp[:])
                swp = qn_p.tile([128, ST], BF16, tag="swp")
                ngrp = 2 if blk < 2 else 1
                for g in range(ngrp):
                    nc.sync.dma_start(out=swp[g * 64:g * 64 + 32, :],
                                      in_=raw[g * 64 + 32:g * 64 + 64, :])
                    nc.sync.dma_start(out=swp[g * 64 + 32:g * 64 + 64, :],
                                      in_=raw[g * 64:g * 64 + 32, :])
                ts = qn_p.tile([128, ST], BF16, tag="ts")
                if blk < 2:
                    dst = (qta if blk == 0 else qtb)
                    nc.vector.tensor_tensor(dst[:], pp[:], cos_sb[:, tsl], MUL)
                    nc.vector.tensor_tensor(ts[:], swp[:], sin_sb[:, tsl], MUL)
                    nc.vector.tensor_tensor(dst[:], dst[:], ts[:], ADD)
                else:
                    kd = kt_res[0:64, ssl]
                    nc.vector.tensor_tensor(kd, pp[0:64, :],
                                            cos_sb[0:64, tsl], MUL)
                    nc.vector.tensor_tensor(ts[0:64, :], swp[0:64, :],
                                            sin_sb[0:64, tsl], MUL)
                    nc.vector.tensor_tensor(kd, kd, ts[0:64, :], ADD)
                    # V.T rows -> natural [kv, d] into the resident
                    for ptb in range(4):
                        nc.sync.dma_start(
                            out=vp_res[:, b * NKV + sti * 4 + ptb, 0:64],
                            in_=raw[64:128, ptb * 128:(ptb + 1) * 128],
                            transpose=True)
            nc.sync.dma_start(out=kt2_res[64:128, ssl], in_=kt_res[0:64, ssl])

            ob = ob_p.tile([128, 4, HPC * D], F32)

            # ---- attention for the 4 heads of this q-tile ----
            for h in range(HPC):
                qh = (qta if h < 2 else qtb)[:, :]
                ktsrc = kt_res if h % 2 == 0 else kt2_res

                def kt(j):
                    return ktsrc[:, b * S + j * 128:b * S + (j + 1) * 128]

                def vp(j):
                    return vp_res[:, b * NKV + j, :]

                cxt = cx_ps.tile([128, ST], F32, tag="cxt")
                first = True
                # full sub-diagonal blocks, two kv blocks per PSUM tile
                for jp in range(0, 4 * sti, 2):
                    sc = sc_ps.tile([128, 2, ST], F32, tag="sc")
                    for jj in (0, 1):
                        nc.tensor.matmul(sc[:, jj, :], kt(jp + jj), qh[:, :],
                                         start=True, stop=True)
                    psb = p_p.tile([128, 2, ST], BF16, tag="psb")
                    nc.scalar.activation(psb[:], sc[:], EXP, scale=0.125)
                    for jj in (0, 1):
                        nc.tensor.matmul(
                            cxt[0:80, :], vp(jp + jj), psb[:, jj, :],
                            start=first, stop=False)
                        first = False
                # diagonal strips r=0..3 (kv block 4*sti+r vs q cols
                # 128r:512), packed two per PSUM tile
                j0 = 4 * sti
                scd = sc_ps.tile([128, 2 * ST], F32, tag="sc")
                nc.tensor.matmul(scd[:, 0:512], kt(j0), qh[:, :],
                                 start=True, stop=True)
                nc.tensor.matmul(scd[:, 512:896], kt(j0 + 1), qh[:, 128:512],
                                 start=True, stop=True)
                nc.vector.copy_predicated(scd[:, 0:128], mask_sb[:], neg_sb[:])
                nc.vector.copy_predicated(scd[:, 512:640], mask_sb[:],
                                          neg_sb[:])
                psbd = p_p.tile([128, 2 * ST], BF16, tag="psb")
                nc.scalar.activation(psbd[:, 0:896], scd[:, 0:896], EXP,
                                     scale=0.125)
                nc.tensor.matmul(cxt[0:80, :], vp(j0), psbd[:, 0:512],
                                 start=first, stop=False)
                nc.tensor.matmul(cxt[0:80, 128:512], vp(j0 + 1),
                                 psbd[:, 512:896], start=False, stop=False)

                scd2 = sc_ps.tile([128, 2 * ST], F32, tag="sc")
                nc.tensor.matmul(scd2[:, 0:256], kt(j0 + 2), qh[:, 256:512],
                                 start=True, stop=True)
                nc.tensor.matmul(scd2[:, 256:384], kt(j0 + 3), qh[:, 384:512],
                                 start=True, stop=True)
                nc.vector.copy_predicated(scd2[:, 0:128], mask_sb[:],
                                          neg_sb[:])
                nc.vector.copy_predicated(scd2[:, 256:384], mask_sb[:],
                                          neg_sb[:])
                psbd2 = p_p.tile([128, 2 * ST], BF16, tag="psb")
                nc.scalar.activation(psbd2[:, 0:384], scd2[:, 0:384], EXP,
                                     scale=0.125)
                nc.tensor.matmul(cxt[0:80, 256:512], vp(j0 + 2),
                                 psbd2[:, 0:256], start=False, stop=False)
                nc.tensor.matmul(cxt[0:80, 384:512], vp(j0 + 3),
                                 psbd2[:, 256:384], start=False, stop=True)

                # ---- finalize: ctx.T -> [q, 80] via PE transpose ----
                # one fi tile per head; the 4 q-block transposes land in
                # disjoint subtiles so they issue back-to-back on the PE
                cxs = cxs_p.tile([80, ST], BF16)
                nc.vector.tensor_copy(cxs[:], cxt[0:80, :])
                fi = fi_ps.tile([128, 4, 80], BF16, tag="fi")
                for qq in range(4):
                    nc.tensor.transpose(fi[:, qq, :],
                                        cxs[:, qq * 128:(qq + 1) * 128],
                                        ident[0:80, 0:80])
                    rv = fo_p.tile([128, 1], F32, tag="rv")
                    nc.vector.reciprocal(rv[:], fi[:, qq, 64:65])
                    nc.vector.tensor_scalar_mul(
                        ob[:, qq, h * 64:(h + 1) * 64], fi[:, qq, 0:64], rv[:])

            nc.sync.dma_start(
                out=out_d[st * ST:(st + 1) * ST, :].rearrange(
                    "(q p) n -> p q n", p=128),
                in_=ob[:])
    return nc


_NC_CACHE = None

# head-dim permutation shared by Q and K: evens then odds (scores are
# invariant; turns interleaved-pair RoPE into a 32-row rotate-half)
_PERM64 = np.concatenate([np.arange(0, D, 2), np.arange(1, D, 2)])


def _host_consts():
    freqs = 1.0 / (10000.0 ** (np.arange(32, dtype=np.float64) * 2 / D))
    ang = freqs[:, None] * np.arange(S, dtype=np.float64)[None, :]  # (32, S)
    cos32 = np.cos(ang)
    sin32 = np.sin(ang)
    rr = np.arange(128) % D
    p32 = rr % 32
    cosT = cos32[p32, :].astype(np.float32)                    # (128, S)
    sgn = np.where(rr < 32, -1.0, 1.0)[:, None]
    sinT = (sin32[p32, :] * sgn).astype(np.float32)            # (128, S)
    kv, qq = np.meshgrid(np.arange(128), np.arange(128), indexing="ij")
    maskinv = (kv > qq).astype(np.uint8)                       # 1 = forbidden
    ident = np.eye(128, dtype=np.float32).astype(ml_dtypes.bfloat16)
    return cosT, sinT, maskinv, ident


def _in_maps(x, Wq, Wk, Wv):
    x = np.asarray(x, dtype=np.float32).reshape(B * S, DIN)
    xt = np.ascontiguousarray(x.T).astype(ml_dtypes.bfloat16)
    Wq = np.asarray(Wq, dtype=np.float32)
    Wk = np.asarray(Wk, dtype=np.float32)
    Wv = np.asarray(Wv, dtype=np.float32)
    # permute head-dims of Q and K weights (evens first) for rotate-half rope
    Wq = Wq.reshape(DIN, 32, D)[:, :, _PERM64].reshape(DIN, 32 * D)
    Wk = Wk.reshape(DIN, 8, D)[:, :, _PERM64].reshape(DIN, 8 * D)
    cosT, sinT, maskinv, ident = _host_consts()

    in_maps = []
    for k in range(NCORES):
        w_all = np.hstack([
            Wq[:, k * 256:(k + 1) * 256],
            Wk[:, k * 64:(k + 1) * 64],
            Wv[:, k * 64:(k + 1) * 64],
        ]).astype(ml_dtypes.bfloat16)
        in_maps.append({
            "xt": xt, "w": np.ascontiguousarray(w_all),
            "cost": cosT, "sint": sinT, "mask": maskinv, "ident": ident,
        })
    return in_maps


def _run(in_maps, **kwargs):
    global _NC_CACHE
    if _NC_CACHE is None:
        _NC_CACHE = build_bass()
        _NC_CACHE.finalize()
    return run_bass_kernel_spmd(_NC_CACHE, in_maps, list(range(NCORES)),
                                **kwargs)


def kernel(x, Wq, Wk, Wv):
    res = _run(_in_maps(x, Wq, Wk, Wv))
    out = np.concatenate([res.results[k]["out"] for k in range(NCORES)], axis=1)
    return out.reshape(B, S, 32 * D)
